# revision 24
# baseline (speedup 1.0000x reference)
"""Trainium2 Bass kernel for nn_MixtureOfUnits (MoE routing + grouped conv).

Strategy (data-parallel over batch, B == n_cores == 8):
  * Host: run the tiny router MLP (replicated, negligible FLOPs) to get the
    top-2 unit per batch element; this decides the sharding: core b receives
    batch b's image plus ONLY the two selected units' conv weights.
  * Device (per core): the heavy compute.  Only 2 of 8 units have nonzero
    input, so instead of the reference's dense 8-unit grouped conv we run
    2 convs worth of matmuls.  The gate softmax needs mean(conv_u + bias)
    over (c,h,w); that mean is computed EXACTLY on device from windowed
    sums of x (T statistics) dotted with per-unit weight column sums:
        mean_chw(conv(x, w_u)) = sum_{ci,tap} T[ci,tap]*Wsum_u[ci,tap] / (C*H*W)
    so the gate is known BEFORE the conv runs.  The conv weights are then
    pre-scaled by the gate on device and both units accumulate into the same
    PSUM chunk together with the gated bias (K=1 matmul), producing the final
    combined output directly -> streamed out via DMA from PSUM.
  * Host: assembles final + the cheap auxiliary outputs (cls_logits, l2_reg,
    top_i, lb_loss) exactly as the reference computes them.
"""

import os
import sys

for _p in ("/opt/trn_rl_repo", "/root/.axon_site/_ro/trn_rl_repo"):
    if os.path.isdir(_p) and _p not in sys.path:
        sys.path.insert(0, _p)

import numpy as np
import ml_dtypes

B, C, H, W = 8, 128, 64, 64
U, TOP_K, L, HID = 8, 2, 10, 128
HP, WP = H + 2, W + 2          # padded spatial dims
HWP = HP * WP
HW = H * W
N_CORES = 8
CHUNK_ROWS = 8                 # output rows per PSUM chunk (8*64 = 512 cols)
N_CHUNKS = H // CHUNK_ROWS

LAST_RESULTS = None  # BassKernelResults of the most recent device run (for profiling)

_CACHED_NC = None


def _legalize_sync_waits(nc):
    """This walrus build encodes at most ONE sync-wait per instruction.
    Tile's scheduler (and its exit drain) can attach several.  Split: for
    every instruction carrying k > 1 waits, insert k-1 NOPs on the same
    engine immediately before it, each carrying one of the extra waits.
    Same-engine program order preserves the happens-before guarantee."""
    from concourse import mybir

    counter = [0]
    for fn in nc.m.functions:
        for blk in fn.blocks:
            insts = list(blk.instructions)
            new_list = []
            changed = False
            for ins in insts:
                si = ins.sync_info
                if si is not None and len(si.on_wait) > 1:
                    waits = list(si.on_wait)
                    for w in waits[:-1]:
                        counter[0] += 1
                        nop = mybir.InstNoOp(
                            name=f"waitsplit-{counter[0]}", ins=[], outs=[]
                        )
                        nop.engine = ins.engine
                        nop.sync_info = mybir.SyncInfo(on_wait=[w], on_update=[])
                        nc.register_instruction(nop, overwrite=True)
                        new_list.append(nop)
                    ins.sync_info = mybir.SyncInfo(
                        on_wait=[waits[-1]], on_update=list(si.on_update)
                    )
                    changed = True
                new_list.append(ins)
            if changed:
                blk.instructions = new_list


def _build_bass():
    """Build the single-core SPMD Bass program (same program on all 8 cores;
    per-core behaviour comes entirely from the per-core input tensors)."""
    import concourse.bass as bass
    import concourse.tile as tile
    from concourse import mybir
    from concourse.masks import make_identity

    dt = mybir.dt
    f32, bf16 = dt.float32, dt.bfloat16
    AF = mybir.ActivationFunctionType
    Alu = mybir.AluOpType
    AX = mybir.AxisListType.X

    nc = bass.Bass()

    xpad_d = nc.dram_tensor("xpad", [C, HWP], bf16, kind="ExternalInput")
    wsel_d = nc.dram_tensor("wsel", [C, 2 * 9 * C], bf16, kind="ExternalInput")
    wsum_d = nc.dram_tensor("wsum", [C, 18], f32, kind="ExternalInput")
    base8_d = nc.dram_tensor("base8", [C, 8], f32, kind="ExternalInput")
    cbt_d = nc.dram_tensor("cbT", [C, 8], f32, kind="ExternalInput")
    fin_d = nc.dram_tensor("final", [C, HW], f32, kind="ExternalOutput")

    with tile.TileContext(nc) as tc:
        with (
            tc.tile_pool(name="main", bufs=1) as pool,
            tc.tile_pool(name="stage", bufs=4) as stage_pool,
            tc.tile_pool(name="cpsum", bufs=8, space="PSUM") as cpsum,
            tc.tile_pool(name="dram", bufs=1, space="DRAM") as dpool,
        ):
            X = pool.tile([C, HWP], bf16)
            Wt = pool.tile([C, 2 * 9 * C], bf16)
            Ws = pool.tile([C, 18], f32)
            B8 = pool.tile([C, 8], f32)
            CbT = pool.tile([C, 8], f32)
            # Weights first (they gate the conv start); x split between the
            # sync (HWDGE) and gpsimd (SWDGE) issue engines so the ~0.7us
            # per-DMA issue costs don't serialize on one engine.
            nc.sync.dma_start(Wt[:], wsel_d[:])
            ROWQ = (17, 33, 50, 66)  # row-aligned x quarters
            prev = 0
            for qi, rend in enumerate(ROWQ):
                lo, hi = prev * WP, rend * WP
                eng = nc.sync if qi % 2 == 0 else nc.gpsimd
                eng.dma_start(X[:, lo:hi], xpad_d[:, lo:hi])
                prev = rend
            nc.gpsimd.dma_start(Ws[:], wsum_d[:])
            nc.gpsimd.dma_start(B8[:], base8_d[:])
            nc.gpsimd.dma_start(CbT[:], cbt_d[:])

            x3 = X[:].rearrange("p (h w) -> p h w", w=WP)

            # ---- T statistics: T[:, 3*ky+kx] = windowed 64x64 sum of xpad ----
            # One full reduce + algebraic edge corrections (cheap [C, small] ops)
            # instead of 12 serial DVE reduces.
            rfull = pool.tile([C, HP], f32)      # rfull[r] = sum_c xpad[:, r, c]
            prev = 0
            for rend in ROWQ:
                nc.vector.reduce_sum(
                    rfull[:, prev:rend], x3[:, prev:rend, :], axis=AX
                )
                prev = rend
            # rsw[:, kx, r] = sum over cols [kx, kx+64) = rfull - 2 edge columns
            rsw = pool.tile([C, 3, HP], f32)
            colpair = ((64, 65), (0, 65), (0, 1))  # excluded cols per kx
            for kx in range(3):
                c1, c2 = colpair[kx]
                nc.vector.tensor_tensor(
                    rsw[:, kx, :], rfull[:], x3[:, :, c1], Alu.subtract
                )
                nc.vector.tensor_tensor(
                    rsw[:, kx, :], rsw[:, kx, :], x3[:, :, c2], Alu.subtract
                )
            # cfull[kx] = sum over all rows of rsw[kx]
            cfull = pool.tile([C, 3], f32)
            nc.vector.reduce_sum(cfull[:], rsw[:], axis=AX)
            # T[ky*3+kx] = cfull[kx] - 2 excluded rows of rsw[kx]
            T = pool.tile([C, 9], f32)
            for ky in range(3):
                r1, r2 = colpair[ky]
                nc.vector.tensor_tensor(
                    T[:, ky * 3 : ky * 3 + 3], cfull[:], rsw[:, :, r1], Alu.subtract
                )
                nc.vector.tensor_tensor(
                    T[:, ky * 3 : ky * 3 + 3],
                    T[:, ky * 3 : ky * 3 + 3],
                    rsw[:, :, r2],
                    Alu.subtract,
                )

            # ---- per-selected-unit conv-output totals (before /(C*H*W)) ----
            ws3 = Ws[:].rearrange("p (u t) -> p u t", t=9)
            t3 = T[:, None, :].to_broadcast((C, 2, 9))
            wd = pool.tile([C, 2, 9], f32)
            nc.vector.tensor_tensor(wd[:], ws3, t3, Alu.mult)
            pm = pool.tile([C, 2], f32)
            nc.vector.reduce_sum(pm[:], wd[:], axis=AX)

            # partition-sum of pm via ONE DRAM bounce, read back transposed
            # AND partition-broadcast; the softmax then runs redundantly on
            # every partition so no second (broadcast) bounce is needed.
            pmd = dpool.tile([C, 2], f32)
            nc.sync.dma_start(pmd[:], pm[:])
            pmT = pool.tile([C, 2 * C], f32)
            nc.sync.dma_start(
                pmT[:],
                pmd[:].rearrange("p u -> (p u)")[None, :].to_broadcast((C, 2 * C)),
            )
            msum = pool.tile([C, 2], f32)
            nc.vector.reduce_sum(
                msum[:], pmT[:].rearrange("p (q u) -> p u q", u=2), axis=AX
            )

            # ---- pre-softmax vector (permuted unit order: sel0, sel1, rest) ----
            pre8 = pool.tile([C, 8], f32)
            nc.vector.tensor_copy(pre8[:], B8[:])
            nc.vector.tensor_scalar(
                pre8[:, 0:2], msum[:], 1.0 / float(C * HW), None, Alu.mult
            )
            nc.vector.tensor_tensor(pre8[:, 0:2], pre8[:, 0:2], B8[:, 0:2], Alu.add)

            # ---- softmax over the 8 units (identical on every partition) ----
            rmax = pool.tile([C, 1], f32)
            nc.vector.reduce_max(rmax[:], pre8[:], axis=AX)
            nrmax = pool.tile([C, 1], f32)
            nc.vector.tensor_scalar_mul(nrmax[:], rmax[:], -1.0)
            e8 = pool.tile([C, 8], f32)
            nc.scalar.activation(e8[:], pre8[:], AF.Exp, bias=nrmax[:], scale=1.0)
            esum = pool.tile([C, 1], f32)
            nc.vector.reduce_sum(esum[:], e8[:], axis=AX)
            rinv = pool.tile([C, 1], f32)
            nc.vector.reciprocal(rinv[:], esum[:])
            gball = pool.tile([C, 8], f32)
            nc.vector.tensor_scalar_mul(gball[:], e8[:], rinv[:])

            # ---- gated bias: bcomb[c] = sum_u gate[u] * conv_b[perm[u], c] ----
            bct = pool.tile([C, 8], f32)
            nc.vector.tensor_mul(bct[:], CbT[:], gball[:])
            bcomb = pool.tile([C, 1], f32)
            nc.vector.reduce_sum(bcomb[:], bct[:], axis=AX)

            # ---- conv: unit-major, tap-outer so each LDWEIGHTS feeds 8
            #      back-to-back pipelined matmuls (one PSUM bank per chunk).
            #      The last tap is chunk-interleaved with evictions so banks
            #      free as soon as each chunk's accumulation finishes. ----
            c0 = pool.tile([C, HW], f32)   # unit0's gated+biased partial

            def conv_rhs(j, t):
                ky, kx = t // 3, t % 3
                return x3[
                    :,
                    j * CHUNK_ROWS + ky : (j + 1) * CHUNK_ROWS + ky,
                    kx : kx + W,
                ]

            # unit 0
            ps_tiles = [
                cpsum.tile([C, CHUNK_ROWS * W], f32, tag="conv", name=f"ps0_{_j}")
                for _j in range(N_CHUNKS)
            ]
            for t in range(8):
                w_ap = Wt[:, t * C : (t + 1) * C]
                for j in range(N_CHUNKS):
                    nc.tensor.matmul(
                        ps_tiles[j][:], w_ap, conv_rhs(j, t), start=(t == 0), stop=False
                    )
            w_ap = Wt[:, 8 * C : 9 * C]
            for j in range(N_CHUNKS):
                nc.tensor.matmul(
                    ps_tiles[j][:], w_ap, conv_rhs(j, 8), start=False, stop=True
                )
                # evict: c0 = ps * g0 + bcomb (alternate engines)
                dst = c0[:, j * 512 : (j + 1) * 512]
                if j % 2 == 0:
                    nc.scalar.activation(
                        dst, ps_tiles[j][:], AF.Identity,
                        bias=bcomb[:], scale=gball[:, 0:1],
                    )
                else:
                    nc.vector.tensor_scalar(
                        dst, ps_tiles[j][:], gball[:, 0:1], bcomb[:],
                        Alu.mult, Alu.add,
                    )

            # unit 1
            ps_tiles2 = [
                cpsum.tile([C, CHUNK_ROWS * W], f32, tag="conv", name=f"ps1_{_j}")
                for _j in range(N_CHUNKS)
            ]
            for t in range(8):
                w_ap = Wt[:, (9 + t) * C : (10 + t) * C]
                for j in range(N_CHUNKS):
                    nc.tensor.matmul(
                        ps_tiles2[j][:], w_ap, conv_rhs(j, t), start=(t == 0), stop=False
                    )
            w_ap = Wt[:, 17 * C : 18 * C]
            for j in range(N_CHUNKS):
                nc.tensor.matmul(
                    ps_tiles2[j][:], w_ap, conv_rhs(j, 8), start=False, stop=True
                )
                # final = ps1 * g1 + c0, streamed out per chunk
                stage = stage_pool.tile([C, CHUNK_ROWS * W], f32, tag="stage")
                if j % 2 == 0:
                    nc.vector.scalar_tensor_tensor(
                        stage[:],
                        ps_tiles2[j][:],
                        gball[:, 1:2],
                        c0[:, j * 512 : (j + 1) * 512],
                        Alu.mult,
                        Alu.add,
                    )
                else:
                    # ACT does the PSUM read+scale, the (otherwise idle)
                    # GpSimd engine does the SBUF+SBUF add
                    nc.scalar.mul(stage[:], ps_tiles2[j][:], gball[:, 1:2])
                    nc.gpsimd.tensor_add(
                        stage[:], stage[:], c0[:, j * 512 : (j + 1) * 512]
                    )
                nc.sync.dma_start(
                    fin_d[:, j * CHUNK_ROWS * W : (j + 1) * CHUNK_ROWS * W], stage[:]
                )

    _legalize_sync_waits(nc)
    return nc


def _get_nc():
    global _CACHED_NC
    if _CACHED_NC is None:
        _CACHED_NC = _build_bass()
    return _CACHED_NC


def _router(x, W1, b1, W2, b2, W3, b3, W4, b4, Wu, bu, Wc, bc):
    """Replicate the reference's router verbatim with jax on the DEFAULT
    backend, so the routing decisions (and the cheap auxiliary outputs)
    match the harness's reference bit-for-bit in this environment.  The
    router is tiny (a few [8,128] matmuls) — negligible vs the convs."""
    import jax
    import jax.numpy as jnp

    x = jnp.asarray(x)
    Bt = x.shape[0]
    pooled = x.mean(axis=(2, 3))
    h = jax.nn.relu(pooled @ jnp.asarray(W1).T + jnp.asarray(b1))
    h = jax.nn.relu(h @ jnp.asarray(W2).T + jnp.asarray(b2))
    h = jax.nn.relu(h @ jnp.asarray(W3).T + jnp.asarray(b3))
    feat = h @ jnp.asarray(W4).T + jnp.asarray(b4)
    unit_scores = feat @ jnp.asarray(Wu).T + jnp.asarray(bu)
    cls_logits = feat @ jnp.asarray(Wc).T + jnp.asarray(bc)
    noise = (
        jax.random.normal(jax.random.key(42), unit_scores.shape, unit_scores.dtype)
        * 0.01
    )
    unit_scores = unit_scores + noise
    top_v, top_i = jax.lax.top_k(unit_scores, TOP_K)
    unit_usage = jnp.zeros((U,), x.dtype).at[top_i.reshape(-1)].add(1.0) / (
        Bt * TOP_K
    )
    lb_loss = ((unit_usage - unit_usage.mean()) ** 2).mean()
    l2_reg = 0.01 * sum(
        jnp.linalg.norm(p)
        for p in [W1, b1, W2, b2, W3, b3, W4, b4, Wu, bu, Wc, bc]
    )
    return (
        np.asarray(cls_logits),
        np.asarray(l2_reg),
        np.asarray(top_i, np.int32),
        np.asarray(lb_loss),
    )


def kernel(x, W1, b1, W2, b2, W3, b3, W4, b4, Wu, bu, Wc, bc, conv_w, conv_b):
    global LAST_RESULTS
    cls_logits, l2_reg, top_i, lb_loss = _router(
        x, W1, b1, W2, b2, W3, b3, W4, b4, Wu, bu, Wc, bc
    )
    x = np.asarray(x, np.float32)

    # ---------- host: shard prep ----------
    w5 = np.asarray(conv_w, np.float32).reshape(U, C, C, 3, 3)      # [u, co, ci, ky, kx]
    cb8 = np.asarray(conv_b, np.float32).reshape(U, C)              # [u, c]
    cb_mean = cb8.mean(axis=1, dtype=np.float32)                    # [u]

    in_maps = []
    for b in range(B):
        i0, i1 = int(top_i[b, 0]), int(top_i[b, 1])
        rest = [u for u in range(U) if u not in (i0, i1)]
        perm = [i0, i1] + rest

        xp = np.zeros((C, HP, WP), np.float32)
        xp[:, 1 : H + 1, 1 : W + 1] = x[b]
        xpad = xp.reshape(C, HWP).astype(ml_dtypes.bfloat16)

        # wsel[ci, u*1152 + t*128 + co] = w5[sel_u, co, ci, ky, kx]
        wsel = (
            w5[[i0, i1]]                       # [2, co, ci, ky, kx]
            .transpose(2, 0, 3, 4, 1)          # [ci, u, ky, kx, co]
            .reshape(C, 2 * 9 * C)
            .astype(ml_dtypes.bfloat16)
        )
        # wsum[ci, u*9 + t] = sum_co w5[sel_u, co, ci, ky, kx]
        wsum = (
            w5[[i0, i1]].sum(axis=1, dtype=np.float32)  # [2, ci, ky, kx]
            .transpose(1, 0, 2, 3)
            .reshape(C, 18)
            .astype(np.float32)
        )
        base8 = np.ascontiguousarray(
            np.broadcast_to(cb_mean[perm].reshape(1, 8), (C, 8))
        )
        cbT = np.ascontiguousarray(cb8[perm].T)         # [c, 8]

        in_maps.append(
            {
                "xpad": xpad,
                "wsel": np.ascontiguousarray(wsel),
                "wsum": np.ascontiguousarray(wsum),
                "base8": base8,
                "cbT": cbT,
            }
        )

    # ---------- device run ----------
    from concourse.bass_utils import run_bass_kernel_spmd

    nc = _get_nc()
    trace = bool(int(os.environ.get("MOE_KERNEL_TRACE", "0")))
    res = run_bass_kernel_spmd(
        nc, in_maps, list(range(N_CORES)), trace=trace
    )
    LAST_RESULTS = res

    final = np.stack(
        [res.results[b]["final"].reshape(C, H, W) for b in range(B)]
    ).astype(np.float32)

    return (final, cls_logits, l2_reg, top_i, lb_loss)


# revision 26
# speedup vs baseline: 1.0514x; 1.0514x over previous
"""Trainium2 Bass kernel for nn_MixtureOfUnits (MoE routing + grouped conv).

Strategy (data-parallel over batch, B == n_cores == 8):
  * Host: run the tiny router MLP (replicated, negligible FLOPs) to get the
    top-2 unit per batch element; this decides the sharding: core b receives
    batch b's image plus ONLY the two selected units' conv weights.
  * Device (per core): the heavy compute.  Only 2 of 8 units have nonzero
    input, so instead of the reference's dense 8-unit grouped conv we run
    2 convs worth of matmuls.  The gate softmax needs mean(conv_u + bias)
    over (c,h,w); that mean is computed EXACTLY on device from windowed
    sums of x (T statistics) dotted with per-unit weight column sums:
        mean_chw(conv(x, w_u)) = sum_{ci,tap} T[ci,tap]*Wsum_u[ci,tap] / (C*H*W)
    so the gate is known BEFORE the conv runs.  The conv weights are then
    pre-scaled by the gate on device and both units accumulate into the same
    PSUM chunk together with the gated bias (K=1 matmul), producing the final
    combined output directly -> streamed out via DMA from PSUM.
  * Host: assembles final + the cheap auxiliary outputs (cls_logits, l2_reg,
    top_i, lb_loss) exactly as the reference computes them.
"""

import os
import sys

for _p in ("/opt/trn_rl_repo", "/root/.axon_site/_ro/trn_rl_repo"):
    if os.path.isdir(_p) and _p not in sys.path:
        sys.path.insert(0, _p)

import numpy as np
import ml_dtypes

B, C, H, W = 8, 128, 64, 64
U, TOP_K, L, HID = 8, 2, 10, 128
HP, WP = H + 2, W + 2          # padded spatial dims
HWP = HP * WP
HW = H * W
N_CORES = 8
CHUNK_ROWS = 8                 # output rows per PSUM chunk (8*64 = 512 cols)
N_CHUNKS = H // CHUNK_ROWS

LAST_RESULTS = None  # BassKernelResults of the most recent device run (for profiling)

_CACHED_NC = None


def _legalize_sync_waits(nc):
    """This walrus build encodes at most ONE sync-wait per instruction.
    Tile's scheduler (and its exit drain) can attach several.  Split: for
    every instruction carrying k > 1 waits, insert k-1 NOPs on the same
    engine immediately before it, each carrying one of the extra waits.
    Same-engine program order preserves the happens-before guarantee."""
    from concourse import mybir

    counter = [0]
    for fn in nc.m.functions:
        for blk in fn.blocks:
            insts = list(blk.instructions)
            new_list = []
            changed = False
            for ins in insts:
                si = ins.sync_info
                if si is not None and len(si.on_wait) > 1:
                    waits = list(si.on_wait)
                    for w in waits[:-1]:
                        counter[0] += 1
                        nop = mybir.InstNoOp(
                            name=f"waitsplit-{counter[0]}", ins=[], outs=[]
                        )
                        nop.engine = ins.engine
                        nop.sync_info = mybir.SyncInfo(on_wait=[w], on_update=[])
                        nc.register_instruction(nop, overwrite=True)
                        new_list.append(nop)
                    ins.sync_info = mybir.SyncInfo(
                        on_wait=[waits[-1]], on_update=list(si.on_update)
                    )
                    changed = True
                new_list.append(ins)
            if changed:
                blk.instructions = new_list


def _build_bass():
    """Build the single-core SPMD Bass program (same program on all 8 cores;
    per-core behaviour comes entirely from the per-core input tensors)."""
    import concourse.bass as bass
    import concourse.tile as tile
    from concourse import mybir
    from concourse.masks import make_identity

    dt = mybir.dt
    f32, bf16 = dt.float32, dt.bfloat16
    AF = mybir.ActivationFunctionType
    Alu = mybir.AluOpType
    AX = mybir.AxisListType.X

    nc = bass.Bass()

    xpad_d = nc.dram_tensor("xpad", [C, HWP], bf16, kind="ExternalInput")
    wsel_d = nc.dram_tensor("wsel", [C, 2 * 9 * C], bf16, kind="ExternalInput")
    wsum_d = nc.dram_tensor("wsum", [C, 18], f32, kind="ExternalInput")
    base8_d = nc.dram_tensor("base8", [C, 8], f32, kind="ExternalInput")
    cbt_d = nc.dram_tensor("cbT", [C, 8], f32, kind="ExternalInput")
    fin_d = nc.dram_tensor("final", [C, HW], f32, kind="ExternalOutput")

    with tile.TileContext(nc) as tc:
        with (
            tc.tile_pool(name="main", bufs=1) as pool,
            tc.tile_pool(name="stage", bufs=4) as stage_pool,
            tc.tile_pool(name="cpsum", bufs=8, space="PSUM") as cpsum,
            tc.tile_pool(name="dram", bufs=1, space="DRAM") as dpool,
        ):
            X = pool.tile([C, HWP], bf16)
            Wt = pool.tile([C, 2 * 9 * C], bf16)
            Ws = pool.tile([C, 18], f32)
            B8 = pool.tile([C, 8], f32)
            CbT = pool.tile([C, 8], f32)
            # Weights first (they gate the conv start); x split between the
            # sync (HWDGE) and gpsimd (SWDGE) issue engines so the ~0.7us
            # per-DMA issue costs don't serialize on one engine.
            nc.sync.dma_start(Wt[:], wsel_d[:])
            ROWQ = (17, 33, 50, 66)  # row-aligned x quarters
            prev = 0
            for qi, rend in enumerate(ROWQ):
                lo, hi = prev * WP, rend * WP
                eng = nc.sync if qi % 2 == 0 else nc.gpsimd
                eng.dma_start(X[:, lo:hi], xpad_d[:, lo:hi])
                prev = rend
            nc.gpsimd.dma_start(Ws[:], wsum_d[:])
            nc.gpsimd.dma_start(B8[:], base8_d[:])
            nc.gpsimd.dma_start(CbT[:], cbt_d[:])

            x3 = X[:].rearrange("p (h w) -> p h w", w=WP)

            # ---- T statistics: T[:, 3*ky+kx] = windowed 64x64 sum of xpad ----
            # One full reduce + algebraic edge corrections (cheap [C, small] ops)
            # instead of 12 serial DVE reduces.
            rfull = pool.tile([C, HP], f32)      # rfull[r] = sum_c xpad[:, r, c]
            prev = 0
            for rend in ROWQ:
                nc.vector.reduce_sum(
                    rfull[:, prev:rend], x3[:, prev:rend, :], axis=AX
                )
                prev = rend
            # rsw[:, kx, r] = sum over cols [kx, kx+64) = rfull - 2 edge columns
            rsw = pool.tile([C, 3, HP], f32)
            colpair = ((64, 65), (0, 65), (0, 1))  # excluded cols per kx
            for kx in range(3):
                c1, c2 = colpair[kx]
                nc.vector.tensor_tensor(
                    rsw[:, kx, :], rfull[:], x3[:, :, c1], Alu.subtract
                )
                nc.vector.tensor_tensor(
                    rsw[:, kx, :], rsw[:, kx, :], x3[:, :, c2], Alu.subtract
                )
            # cfull[kx] = sum over all rows of rsw[kx]
            cfull = pool.tile([C, 3], f32)
            nc.vector.reduce_sum(cfull[:], rsw[:], axis=AX)
            # T[ky*3+kx] = cfull[kx] - 2 excluded rows of rsw[kx]
            T = pool.tile([C, 9], f32)
            for ky in range(3):
                r1, r2 = colpair[ky]
                nc.vector.tensor_tensor(
                    T[:, ky * 3 : ky * 3 + 3], cfull[:], rsw[:, :, r1], Alu.subtract
                )
                nc.vector.tensor_tensor(
                    T[:, ky * 3 : ky * 3 + 3],
                    T[:, ky * 3 : ky * 3 + 3],
                    rsw[:, :, r2],
                    Alu.subtract,
                )

            # ---- per-selected-unit conv-output totals (before /(C*H*W)) ----
            ws3 = Ws[:].rearrange("p (u t) -> p u t", t=9)
            t3 = T[:, None, :].to_broadcast((C, 2, 9))
            wd = pool.tile([C, 2, 9], f32)
            nc.vector.tensor_tensor(wd[:], ws3, t3, Alu.mult)
            pm = pool.tile([C, 2], f32)
            nc.vector.reduce_sum(pm[:], wd[:], axis=AX)

            # partition-sum of pm via ONE DRAM bounce, read back transposed
            # AND partition-broadcast; the softmax then runs redundantly on
            # every partition so no second (broadcast) bounce is needed.
            pmd = dpool.tile([C, 2], f32)
            nc.sync.dma_start(pmd[:], pm[:])
            pmT = pool.tile([C, 2 * C], f32)
            nc.sync.dma_start(
                pmT[:],
                pmd[:].rearrange("p u -> (p u)")[None, :].to_broadcast((C, 2 * C)),
            )
            msum = pool.tile([C, 2], f32)
            nc.vector.reduce_sum(
                msum[:], pmT[:].rearrange("p (q u) -> p u q", u=2), axis=AX
            )

            # ---- pre-softmax vector (permuted unit order: sel0, sel1, rest) ----
            pre8 = pool.tile([C, 8], f32)
            nc.vector.tensor_copy(pre8[:], B8[:])
            nc.vector.tensor_scalar(
                pre8[:, 0:2], msum[:], 1.0 / float(C * HW), None, Alu.mult
            )
            nc.vector.tensor_tensor(pre8[:, 0:2], pre8[:, 0:2], B8[:, 0:2], Alu.add)

            # ---- softmax over the 8 units (identical on every partition) ----
            rmax = pool.tile([C, 1], f32)
            nc.vector.reduce_max(rmax[:], pre8[:], axis=AX)
            nrmax = pool.tile([C, 1], f32)
            nc.vector.tensor_scalar_mul(nrmax[:], rmax[:], -1.0)
            e8 = pool.tile([C, 8], f32)
            nc.scalar.activation(e8[:], pre8[:], AF.Exp, bias=nrmax[:], scale=1.0)
            esum = pool.tile([C, 1], f32)
            nc.vector.reduce_sum(esum[:], e8[:], axis=AX)
            rinv = pool.tile([C, 1], f32)
            nc.vector.reciprocal(rinv[:], esum[:])
            gball = pool.tile([C, 8], f32)
            nc.vector.tensor_scalar_mul(gball[:], e8[:], rinv[:])

            # ---- gated bias: bcomb[c] = sum_u gate[u] * conv_b[perm[u], c] ----
            bct = pool.tile([C, 8], f32)
            nc.vector.tensor_mul(bct[:], CbT[:], gball[:])
            bcomb = pool.tile([C, 1], f32)
            nc.vector.reduce_sum(bcomb[:], bct[:], axis=AX)

            # ---- conv: unit-major, tap-outer so each LDWEIGHTS feeds 8
            #      back-to-back pipelined matmuls (one PSUM bank per chunk).
            #      The last tap is chunk-interleaved with evictions so banks
            #      free as soon as each chunk's accumulation finishes. ----
            def conv_rhs(j, t):
                ky, kx = t // 3, t % 3
                return x3[
                    :,
                    j * CHUNK_ROWS + ky : (j + 1) * CHUNK_ROWS + ky,
                    kx : kx + W,
                ]

            # unit1's weights, pre-scaled by g1/g0 on the fly (ready ~21us,
            # needed ~31us): both units then accumulate into ONE PSUM group
            # and a single gated eviction (x g0, + bcomb) finishes each chunk.
            rg0 = pool.tile([C, 1], f32)
            nc.vector.reciprocal(rg0[:], gball[:, 0:1])
            rg = pool.tile([C, 1], f32)
            nc.vector.tensor_mul(rg[:], gball[:, 1:2], rg0[:])
            Wsc1 = pool.tile([C, 9 * C], bf16)
            nc.vector.tensor_scalar_mul(Wsc1[:], Wt[:, 9 * C :], rg[:])

            ps_tiles = [
                cpsum.tile([C, CHUNK_ROWS * W], f32, tag="conv", name=f"ps_{_j}")
                for _j in range(N_CHUNKS)
            ]
            for t in range(17):
                if t < 9:
                    w_ap = Wt[:, t * C : (t + 1) * C]
                else:
                    w_ap = Wsc1[:, (t - 9) * C : (t - 8) * C]
                for j in range(N_CHUNKS):
                    nc.tensor.matmul(
                        ps_tiles[j][:], w_ap, conv_rhs(j, t % 9), start=(t == 0), stop=False
                    )
            w_ap = Wsc1[:, 8 * C : 9 * C]
            for j in range(N_CHUNKS):
                nc.tensor.matmul(
                    ps_tiles[j][:], w_ap, conv_rhs(j, 8), start=False, stop=True
                )
                # final = ps * g0 + bcomb, streamed out per chunk
                stage = stage_pool.tile([C, CHUNK_ROWS * W], f32, tag="stage")
                if j % 2 == 0:
                    nc.scalar.activation(
                        stage[:], ps_tiles[j][:], AF.Identity,
                        bias=bcomb[:], scale=gball[:, 0:1],
                    )
                else:
                    nc.vector.tensor_scalar(
                        stage[:], ps_tiles[j][:], gball[:, 0:1], bcomb[:],
                        Alu.mult, Alu.add,
                    )
                nc.sync.dma_start(
                    fin_d[:, j * CHUNK_ROWS * W : (j + 1) * CHUNK_ROWS * W], stage[:]
                )

    _legalize_sync_waits(nc)
    return nc


def _get_nc():
    global _CACHED_NC
    if _CACHED_NC is None:
        _CACHED_NC = _build_bass()
    return _CACHED_NC


def _router(x, W1, b1, W2, b2, W3, b3, W4, b4, Wu, bu, Wc, bc):
    """Replicate the reference's router verbatim with jax on the DEFAULT
    backend, so the routing decisions (and the cheap auxiliary outputs)
    match the harness's reference bit-for-bit in this environment.  The
    router is tiny (a few [8,128] matmuls) — negligible vs the convs."""
    import jax
    import jax.numpy as jnp

    x = jnp.asarray(x)
    Bt = x.shape[0]
    pooled = x.mean(axis=(2, 3))
    h = jax.nn.relu(pooled @ jnp.asarray(W1).T + jnp.asarray(b1))
    h = jax.nn.relu(h @ jnp.asarray(W2).T + jnp.asarray(b2))
    h = jax.nn.relu(h @ jnp.asarray(W3).T + jnp.asarray(b3))
    feat = h @ jnp.asarray(W4).T + jnp.asarray(b4)
    unit_scores = feat @ jnp.asarray(Wu).T + jnp.asarray(bu)
    cls_logits = feat @ jnp.asarray(Wc).T + jnp.asarray(bc)
    noise = (
        jax.random.normal(jax.random.key(42), unit_scores.shape, unit_scores.dtype)
        * 0.01
    )
    unit_scores = unit_scores + noise
    top_v, top_i = jax.lax.top_k(unit_scores, TOP_K)
    unit_usage = jnp.zeros((U,), x.dtype).at[top_i.reshape(-1)].add(1.0) / (
        Bt * TOP_K
    )
    lb_loss = ((unit_usage - unit_usage.mean()) ** 2).mean()
    l2_reg = 0.01 * sum(
        jnp.linalg.norm(p)
        for p in [W1, b1, W2, b2, W3, b3, W4, b4, Wu, bu, Wc, bc]
    )
    return (
        np.asarray(cls_logits),
        np.asarray(l2_reg),
        np.asarray(top_i, np.int32),
        np.asarray(lb_loss),
    )


def kernel(x, W1, b1, W2, b2, W3, b3, W4, b4, Wu, bu, Wc, bc, conv_w, conv_b):
    global LAST_RESULTS
    cls_logits, l2_reg, top_i, lb_loss = _router(
        x, W1, b1, W2, b2, W3, b3, W4, b4, Wu, bu, Wc, bc
    )
    x = np.asarray(x, np.float32)

    # ---------- host: shard prep ----------
    w5 = np.asarray(conv_w, np.float32).reshape(U, C, C, 3, 3)      # [u, co, ci, ky, kx]
    cb8 = np.asarray(conv_b, np.float32).reshape(U, C)              # [u, c]
    cb_mean = cb8.mean(axis=1, dtype=np.float32)                    # [u]

    in_maps = []
    for b in range(B):
        i0, i1 = int(top_i[b, 0]), int(top_i[b, 1])
        rest = [u for u in range(U) if u not in (i0, i1)]
        perm = [i0, i1] + rest

        xp = np.zeros((C, HP, WP), np.float32)
        xp[:, 1 : H + 1, 1 : W + 1] = x[b]
        xpad = xp.reshape(C, HWP).astype(ml_dtypes.bfloat16)

        # wsel[ci, u*1152 + t*128 + co] = w5[sel_u, co, ci, ky, kx]
        wsel = (
            w5[[i0, i1]]                       # [2, co, ci, ky, kx]
            .transpose(2, 0, 3, 4, 1)          # [ci, u, ky, kx, co]
            .reshape(C, 2 * 9 * C)
            .astype(ml_dtypes.bfloat16)
        )
        # wsum[ci, u*9 + t] = sum_co w5[sel_u, co, ci, ky, kx]
        wsum = (
            w5[[i0, i1]].sum(axis=1, dtype=np.float32)  # [2, ci, ky, kx]
            .transpose(1, 0, 2, 3)
            .reshape(C, 18)
            .astype(np.float32)
        )
        base8 = np.ascontiguousarray(
            np.broadcast_to(cb_mean[perm].reshape(1, 8), (C, 8))
        )
        cbT = np.ascontiguousarray(cb8[perm].T)         # [c, 8]

        in_maps.append(
            {
                "xpad": xpad,
                "wsel": np.ascontiguousarray(wsel),
                "wsum": np.ascontiguousarray(wsum),
                "base8": base8,
                "cbT": cbT,
            }
        )

    # ---------- device run ----------
    from concourse.bass_utils import run_bass_kernel_spmd

    nc = _get_nc()
    trace = bool(int(os.environ.get("MOE_KERNEL_TRACE", "0")))
    res = run_bass_kernel_spmd(
        nc, in_maps, list(range(N_CORES)), trace=trace
    )
    LAST_RESULTS = res

    final = np.stack(
        [res.results[b]["final"].reshape(C, H, W) for b in range(B)]
    ).astype(np.float32)

    return (final, cls_logits, l2_reg, top_i, lb_loss)
